# revision 18
# baseline (speedup 1.0000x reference)
"""DAV_Block cost-volume kernel for Trainium2 (8 NeuronCores, SPMD).

Computes sim[b,d,h,w] = cosine similarity between 3x3xC patches of q and
warped_feat[..., d]:
    qq  = box3(sum_c q^2);  kk = box3(sum_c wf_d^2);  num = box3(sum_c q*wf_d)
    sim = num / (max(sqrt(qq),eps) * max(sqrt(kk),eps))

Sharding: 8 cores = b(2) x h-quarter(4).  Each core gets a 48-row h-slice
(+1 halo row each side, zeros at global edges) with all C, W, D.

Per-core dataflow (fp32 in, fp32r through the PE):
  partitions = (h-pair, c) = 128
  ACT: sq = wf^2          -> fp32r
  DVE: pr = wf * q_bcast  -> fp32r
  PE : banded ones lhsT [128, 48] performs channel-sum AND the 3-tap h-box
       in one accumulation chain (25 h-pair matmuls per psum bank, M=48).
  Stage [128p = (w-half, h)] : 3-tap w-box via free-dim shifted adds,
       then sqrt + reciprocal_approx_fast normalization.
"""
import numpy as np
from contextlib import ExitStack

import concourse.bass as bass
from concourse import bacc
import concourse.tile as tile
from concourse import mybir
from concourse.bass_utils import run_bass_kernel_spmd

# Problem shape (hardcoded per contest contract)
B, C, H, W, D = 2, 64, 192, 320, 32
NCORES = 8
HQ = 4                 # h-quarters per batch
HOUT = H // HQ         # 48 out rows per core
HIN = HOUT + 2         # 50 input rows (1 halo each side)
NT = HIN // 2          # 25 h-pairs
J0 = HOUT              # center col of the banded weight pattern
GW = 2 * HOUT          # G width: cols [0, 96)
GW2 = GW + HOUT        # + h-box band block for the qq matmul
WBLK = 32              # w columns per main-loop tile
NWB = W // WBLK        # 10
WHALF = W // 2         # 160
FSTG = (WHALF + 2) * D  # stage free size incl. 1 halo col each side: 162*32
FVAL = WHALF * D        # 5120

_NC_CACHE = None


def _build_nc():
    nc = bacc.Bacc(None, target_bir_lowering=False)
    wf_d = nc.declare_dram_parameter("wf", [C, HIN, W, D], mybir.dt.float32, isOutput=False)
    q_d = nc.declare_dram_parameter("q", [C, HIN, W], mybir.dt.float32, isOutput=False)
    g_d = nc.declare_dram_parameter("g", [128, GW2], mybir.dt.float32, isOutput=False)
    o_d = nc.declare_dram_parameter("o", [HOUT, W, D], mybir.dt.float32, isOutput=True)

    f32 = mybir.dt.float32
    f32r = mybir.dt.float32r
    SQ = mybir.ActivationFunctionType.Square

    with ExitStack() as ctx:
        tc = ctx.enter_context(tile.TileContext(nc))
        cpool = ctx.enter_context(tc.tile_pool(name="const", bufs=1))
        spool = ctx.enter_context(tc.tile_pool(name="stage", bufs=1))
        pool = ctx.enter_context(tc.tile_pool(name="work", bufs=6))
        epool = ctx.enter_context(tc.tile_pool(name="elem", bufs=3))

        # ---- constants ----
        g = cpool.tile([128, GW2], f32)
        nc.sync.dma_start(g[:], g_d[:])
        gb = cpool.tile([128, GW2], mybir.dt.bfloat16)
        nc.scalar.copy(gb[:], g[:])

        # q resident: [128p=(hpair,c), t, w]
        q_all = cpool.tile([128, NT, W], f32)
        nc.sync.dma_start(q_all[:], q_d[:].rearrange("c (t hp) w -> hp c t w", hp=2))

        # ---- stage tensors ----
        num_stg = spool.tile([112, FSTG], f32)
        kk_stg = spool.tile([112, FSTG], f32)
        qq_stg = spool.tile([112, WHALF + 2], f32)

        # ---- phase A: qq = box_h(sum_c q^2), mostly OFF the PE ----
        # Every matmul gets a walrus-inserted weight load that ticks a 16-bit
        # semaphore by 64, capping a program at ~1023 matmuls.  The main loop
        # needs exactly 1000, so qq's channel sum runs on ACT+DVE (free-dim
        # tree), and only the cross-partition work (c-half combine + 3-tap
        # h-box) uses the PE: one banded matmul per w-half (2 total).
        # Layout: partitions = (c-half, h), free = (c32, w-half).  walrus
        # forbids TensorTensor with SBUF inputs at different base partitions,
        # so all DVE adds stay within one partition group.
        bf16 = mybir.dt.bfloat16
        CH = C // 2  # 32 channels per partition-group
        for half in range(2):
            with tc.tile_pool(name=f"qq{half}", bufs=1) as qpool, tc.tile_pool(
                name=f"qqp{half}", bufs=1, space="PSUM"
            ) as qpsum:
                q_t = qpool.tile([64 + HIN, CH * WHALF], f32)
                ws = slice(half * WHALF, (half + 1) * WHALF)
                for c2 in range(2):
                    nc.sync.dma_start(
                        q_t[64 * c2 : 64 * c2 + HIN, :].rearrange(
                            "h (c w) -> h c w", c=CH
                        ),
                        q_d[c2 * CH : (c2 + 1) * CH, :, ws].rearrange(
                            "c h w -> h c w"
                        ),
                    )
                qsq = qpool.tile([64 + HIN, CH * WHALF], bf16)
                nc.scalar.activation(qsq[0:HIN, :], q_t[0:HIN, :], SQ)
                nc.scalar.activation(qsq[64 : 64 + HIN, :], q_t[64 : 64 + HIN, :], SQ)
                # pairwise tree-sum over the 32 channels in the free dim
                cur, n = qsq, CH * WHALF
                while n > 2 * WHALF:
                    n //= 2
                    nxt = qpool.tile([64 + HIN, n], bf16, tag=f"tree{n}")
                    for c2 in range(2):
                        ps = slice(64 * c2, 64 * c2 + HIN)
                        nc.vector.tensor_add(
                            nxt[ps, :], cur[ps, 0:n], cur[ps, n : 2 * n]
                        )
                    cur = nxt
                # final level lands in a 128p tile; rows 50:64 zeroed (their
                # band weights are 0, but 0*NaN garbage would still poison)
                qsum2 = qpool.tile([128, WHALF], bf16)
                nc.gpsimd.memset(qsum2[:], 0.0)
                for c2 in range(2):
                    ps = slice(64 * c2, 64 * c2 + HIN)
                    nc.vector.tensor_add(
                        qsum2[ps, :], cur[ps, 0:WHALF], cur[ps, WHALF : 2 * WHALF]
                    )
                # c-half combine + 3-tap h-box in one banded matmul
                qq_acc = qpsum.tile([128, WHALF], f32)
                nc.tensor.matmul(
                    qq_acc[0:HOUT, :],
                    gb[0 : 64 + HIN, GW : GW + HOUT],
                    qsum2[0 : 64 + HIN, :],
                    start=True,
                    stop=True,
                )
                nc.scalar.copy(
                    qq_stg[64 * half : 64 * half + HOUT, 1 : WHALF + 1],
                    qq_acc[0:HOUT, :],
                )

        # ---- phase B: num/kk main loop ----
        with tc.tile_pool(name="mm_psum", bufs=2, space="PSUM") as mpsum:
            for wb in range(NWB):
                acc_num = mpsum.tile([128, 2 * 512], f32, tag="acc_num")
                acc_kk = mpsum.tile([128, 2 * 512], f32, tag="acc_kk")
                for t in range(NT):
                    wf_t = pool.tile([128, WBLK * D], f32, tag="wf")
                    src = (
                        wf_d[:]
                        .rearrange("c (t hp) w d -> t hp c w d", hp=2)[t]
                        [:, :, wb * WBLK : (wb + 1) * WBLK, :]
                    )
                    # alternate HWDGE queues (SP/ACT) — each 16-bit queue
                    # semaphore only has headroom for ~255 DMAs per run
                    dma_eng = nc.sync if (wb * NT + t) % 2 == 0 else nc.scalar
                    dma_eng.dma_start(
                        wf_t[:].rearrange("p (w d) -> p w d", d=D), src
                    )

                    # bf16 PE operands: weight loads tick the 16-bit
                    # weight-load semaphore 16x/load instead of fp32r's 64 —
                    # 1000 fp32r loads overflow it at compile time.
                    sq_t = epool.tile([128, WBLK * D], mybir.dt.bfloat16, tag="sq")
                    nc.scalar.activation(sq_t[:], wf_t[:], SQ)

                    pr_t = epool.tile([128, WBLK * D], mybir.dt.bfloat16, tag="pr")
                    q_b = (
                        q_all[:, t, wb * WBLK : (wb + 1) * WBLK]
                        .unsqueeze(-1)
                        .broadcast_to([128, WBLK, D])
                    )
                    nc.vector.tensor_mul(
                        pr_t[:].rearrange("p (w d) -> p w d", d=D),
                        wf_t[:].rearrange("p (w d) -> p w d", d=D),
                        q_b,
                    )

                    lhsT = gb[:, J0 - 2 * t : J0 - 2 * t + HOUT]
                    first, last = (t == 0), (t == NT - 1)
                    for ch in range(2):
                        sl = slice(512 * ch, 512 * (ch + 1))
                        nc.tensor.matmul(acc_num[0:HOUT, sl], lhsT, pr_t[:, sl],
                                         start=first, stop=last)
                        nc.tensor.matmul(acc_kk[0:HOUT, sl], lhsT, sq_t[:, sl],
                                         start=first, stop=last)

                # evacuate this w-block: psum [48, 1024] -> stage quadrant
                wg, wo = wb // (NWB // 2), (wb % (NWB // 2)) * WBLK
                pbase = 64 * wg
                foff = (1 + wo) * D
                nc.scalar.copy(
                    num_stg[pbase : pbase + HOUT, foff : foff + 1024],
                    acc_num[0:HOUT, :],
                )
                nc.vector.tensor_copy(
                    kk_stg[pbase : pbase + HOUT, foff : foff + 1024],
                    acc_kk[0:HOUT, :],
                )

        # ---- phase C: halos, box-w, normalize ----
        # zero halos at global w edges
        nc.gpsimd.memset(num_stg[0:HOUT, 0:D], 0.0)
        nc.gpsimd.memset(kk_stg[0:HOUT, 0:D], 0.0)
        nc.gpsimd.memset(qq_stg[0:HOUT, 0:1], 0.0)
        nc.gpsimd.memset(num_stg[64 : 64 + HOUT, (WHALF + 1) * D : FSTG], 0.0)
        nc.gpsimd.memset(kk_stg[64 : 64 + HOUT, (WHALF + 1) * D : FSTG], 0.0)
        nc.gpsimd.memset(qq_stg[64 : 64 + HOUT, WHALF + 1 : WHALF + 2], 0.0)
        # interface halos between the two w-halves (cross-quadrant copies)
        nc.scalar.copy(num_stg[0:HOUT, (WHALF + 1) * D : FSTG],
                       num_stg[64 : 64 + HOUT, D : 2 * D])
        nc.scalar.copy(num_stg[64 : 64 + HOUT, 0:D],
                       num_stg[0:HOUT, WHALF * D : (WHALF + 1) * D])
        nc.scalar.copy(kk_stg[0:HOUT, (WHALF + 1) * D : FSTG],
                       kk_stg[64 : 64 + HOUT, D : 2 * D])
        nc.scalar.copy(kk_stg[64 : 64 + HOUT, 0:D],
                       kk_stg[0:HOUT, WHALF * D : (WHALF + 1) * D])
        nc.scalar.copy(qq_stg[0:HOUT, WHALF + 1 : WHALF + 2],
                       qq_stg[64 : 64 + HOUT, 1:2])
        nc.scalar.copy(qq_stg[64 : 64 + HOUT, 0:1],
                       qq_stg[0:HOUT, WHALF : WHALF + 1])

        # box-w (3-tap along w = free-dim shifts by D)
        box_num = spool.tile([112, FVAL], f32)
        box_kk = spool.tile([112, FVAL], f32)
        qq_box = spool.tile([112, WHALF], f32)
        nc.vector.tensor_add(box_num[0:112, :], num_stg[0:112, 0:FVAL],
                             num_stg[0:112, 2 * D : FVAL + 2 * D])
        nc.vector.tensor_add(box_num[0:112, :], box_num[0:112, :],
                             num_stg[0:112, D : FVAL + D])
        nc.vector.tensor_add(box_kk[0:112, :], kk_stg[0:112, 0:FVAL],
                             kk_stg[0:112, 2 * D : FVAL + 2 * D])
        nc.vector.tensor_add(box_kk[0:112, :], box_kk[0:112, :],
                             kk_stg[0:112, D : FVAL + D])
        nc.vector.tensor_add(qq_box[0:112, :], qq_stg[0:112, 0:WHALF],
                             qq_stg[0:112, 2 : WHALF + 2])
        nc.vector.tensor_add(qq_box[0:112, :], qq_box[0:112, :],
                             qq_stg[0:112, 1 : WHALF + 1])

        # normalize: sim = box_num * recip(sqrt(box_kk * qq_box))
        prod = kk_stg  # reuse
        nc.vector.tensor_mul(
            prod[0:112, 0:FVAL].rearrange("p (w d) -> p w d", d=D),
            box_kk[0:112, :].rearrange("p (w d) -> p w d", d=D),
            qq_box[0:112, :].unsqueeze(-1).broadcast_to([112, WHALF, D]),
        )
        s = num_stg  # reuse
        nc.scalar.activation(s[0:112, 0:FVAL], prod[0:112, 0:FVAL],
                             mybir.ActivationFunctionType.Sqrt)
        r = prod  # reuse again
        nc.vector.reciprocal_approx_fast(r[0:112, 0:FVAL], s[0:112, 0:FVAL])
        sim = box_kk  # reuse
        nc.vector.tensor_mul(sim[0:112, :], box_num[0:112, :], r[0:112, 0:FVAL])

        # ---- output ----
        nc.sync.dma_start(
            o_d[:, 0:WHALF, :],
            sim[0:HOUT, :].rearrange("p (w d) -> p w d", d=D),
        )
        nc.sync.dma_start(
            o_d[:, WHALF:W, :],
            sim[64 : 64 + HOUT, :].rearrange("p (w d) -> p w d", d=D),
        )

    nc.compile()
    return nc


def _g_pattern() -> np.ndarray:
    """g[p=(hp*64+c), j] = 1 iff j - J0 in {hp-2, hp-1, hp}; cols GW..GW2
    hold the qq h-box band: partition (c2*64 + r) x col (GW + m) = 1 iff
    m in {r-2, r-1, r} (input local row r covers output rows r-2..r)."""
    g = np.zeros((128, GW2), dtype=np.float32)
    for hp in range(2):
        for dj in (hp - 2, hp - 1, hp):
            j = J0 + dj
            if 0 <= j < GW:
                g[hp * 64 : (hp + 1) * 64, j] = 1.0
    for c2 in range(2):
        for r in range(HIN):
            for m in (r - 2, r - 1, r):
                if 0 <= m < HOUT:
                    g[c2 * 64 + r, GW + m] = 1.0
    return g


def get_nc():
    global _NC_CACHE
    if _NC_CACHE is None:
        _NC_CACHE = _build_nc()
    return _NC_CACHE


def make_in_maps(q: np.ndarray, warped_feat: np.ndarray):
    """Marshal full inputs into 8 per-core input maps."""
    q = np.asarray(q, dtype=np.float32)
    wf = np.asarray(warped_feat, dtype=np.float32)
    g = _g_pattern()
    in_maps = []
    for core in range(NCORES):
        b, j = divmod(core, HQ)
        h0 = j * HOUT - 1          # inclusive, may be -1
        h1 = j * HOUT + HOUT + 1   # exclusive, may be H+1
        lo_pad = 1 if h0 < 0 else 0
        hi_pad = 1 if h1 > H else 0
        hs = slice(h0 + lo_pad, h1 - hi_pad)
        # np.empty + halo-only zeroing: np.zeros here memsets ~1 GB of wf
        # that is immediately overwritten (only 0-1 halo rows need zeros)
        q_c = np.empty((C, HIN, W), dtype=np.float32)
        wf_c = np.empty((C, HIN, W, D), dtype=np.float32)
        if lo_pad:
            q_c[:, 0, :] = 0.0
            wf_c[:, 0, :, :] = 0.0
        if hi_pad:
            q_c[:, HIN - 1, :] = 0.0
            wf_c[:, HIN - 1, :, :] = 0.0
        q_c[:, lo_pad : HIN - hi_pad, :] = q[b][:, hs, :]
        wf_c[:, lo_pad : HIN - hi_pad, :, :] = wf[b][:, hs, :, :]
        in_maps.append({"wf": wf_c, "q": q_c, "g": g})
    return in_maps


def assemble(results) -> np.ndarray:
    out = np.empty((B, D, H, W), dtype=np.float32)
    for core in range(NCORES):
        b, j = divmod(core, HQ)
        o = results[core]["o"]  # [48, 320, 32]
        out[b, :, j * HOUT : (j + 1) * HOUT, :] = o.transpose(2, 0, 1)
    return out


def kernel(q: np.ndarray, warped_feat: np.ndarray) -> np.ndarray:
    nc = get_nc()
    in_maps = make_in_maps(q, warped_feat)
    res = run_bass_kernel_spmd(nc, in_maps, list(range(NCORES)))
    return assemble(res.results)

